# revision 21
# baseline (speedup 1.0000x reference)
"""Local+global sparse attention (T=4096, D=64, window=512, global stride 64)
for Trainium2, one head per NeuronCore (B*H = 8 = n_cores).

Per-head structure (hardcoded for T=4096, D=64), for each of 8 query
superblocks (sb) of 512 queries, scores S^T in [k=128 part, q free] tiles:
  lower tiles j0=0..3 : keys [512(s-1)+128*j0, +128), queries [0, 128*(j0+1))
  diag  tiles u =0..3 : keys [512 s +128*u, +128),    queries [128*u, 512)
  global tile         : ng=8s stride-64 keys k < 512 s, packed two q-halves
                        on partitions [0:ng] / [64:64+ng], 256 q cols each.
Band QK matmuls write shared multi-bank PSUM pair tiles so one wide exp
instruction covers two tiles.  exp is split: ScalarE native Exp for pairs
A (L4+D1), C (L2+D3), and the global tile; DVE runs Schraudolph int16 exp
(round(y*128/ln2 + B) bitcast to bf16) for pairs B (L3+D2) and D (L1+D4).
Causal/window-edge triangles are zeroed post-exp by affine_selects (Pool x7,
DVE x1).  Global-stripe keys inside lower tiles are excluded via ve_low (PV
rhs with those key rows zeroed); their contribution comes from the global
tile.  PV is transposed: out[q=128 part, 65] += E[k, qslice].T @ ve[k, 65]
(65th column of ones = softmax denominator).  Host divides by denominator.
"""

import sys

sys.path.insert(0, "/opt/trn_rl_repo")

from contextlib import ExitStack

import numpy as np
import ml_dtypes

import concourse.bass as bass
import concourse.mybir as mybir
import concourse.tile as tile
from concourse import bacc
from concourse.bass_utils import run_bass_kernel_spmd

f32 = mybir.dt.float32
bf16 = mybir.dt.bfloat16
i16 = mybir.dt.int16
AF = mybir.ActivationFunctionType
ALU = mybir.AluOpType

T, D = 4096, 64
W, GS = 512, 64
NSB = T // 512
SCALE = 1.0 / 8.0
# Schraudolph exp in bf16: round((s*SCALE)*(128/ln2) + SB) as int16 == bf16 bits
SA = (128.0 / float(np.log(2.0))) * SCALE
SB = 16250.40

# E-region offsets (bf16 cols) matching the PSUM pair layout:
# [A3: L4|D1|G 0:1280] [B1: L3|D4 1280:1792] [B2: D2|L1 1792:2304]
# [C: L2|D3 2304:2816]
EOFF_L = {3: 0, 2: 1280, 1: 2304, 0: 1792}
EOFF_D = {0: 512, 1: 1920, 2: 2560, 3: 1664}
EG_OFF = 1024

# superblock processing order: lightest (s=0) drains last
PROC = [1, 2, 3, 4, 5, 6, 7, 0]

TRACE = False
LAST_RESULT = None


def _build_nc():
    nc = bacc.Bacc("TRN2", target_bir_lowering=False, debug=False, num_devices=8)
    kq_d = nc.dram_tensor("kq", [64, 8192], bf16, kind="ExternalInput")
    mm_d = nc.dram_tensor("mm", [128, 256], bf16, kind="ExternalInput")
    ve_d = nc.dram_tensor("ve", [128, 32 * 65], bf16, kind="ExternalInput")
    vel_d = nc.dram_tensor("vel", [128, 32 * 65], bf16, kind="ExternalInput")
    kg_d = nc.dram_tensor("kg", [64, 64], bf16, kind="ExternalInput")
    vg_d = nc.dram_tensor("vg", [128, 65], bf16, kind="ExternalInput")
    o_d = nc.dram_tensor("o", [4, 128, 520], bf16, kind="ExternalOutput")

    with tile.TileContext(nc) as tc:
        with ExitStack() as ctx:
            const = ctx.enter_context(tc.tile_pool(name="const", bufs=1))
            ebp = ctx.enter_context(tc.tile_pool(name="ebp", bufs=3))
            egp = ctx.enter_context(tc.tile_pool(name="egp", bufs=3))
            osb = ctx.enter_context(tc.tile_pool(name="osb", bufs=2))
            ps_a = ctx.enter_context(tc.tile_pool(name="ps_a", bufs=1, space="PSUM"))
            ps_b1 = ctx.enter_context(tc.tile_pool(name="ps_b1", bufs=1, space="PSUM"))
            ps_b2 = ctx.enter_context(tc.tile_pool(name="ps_b2", bufs=1, space="PSUM"))
            ps_c = ctx.enter_context(tc.tile_pool(name="ps_c", bufs=1, space="PSUM"))
            ps_o = ctx.enter_context(tc.tile_pool(name="ps_o", bufs=2, space="PSUM"))

            kq = const.tile([64, 8192], bf16, tag="kq")
            mm_t = const.tile([128, 256], bf16, tag="mm")
            ve = const.tile([128, 32 * 65], bf16, tag="ve")
            vel = const.tile([128, 32 * 65], bf16, tag="vel")
            kg = const.tile([64, 64], bf16, tag="kg")
            vg = const.tile([128, 65], bf16, tag="vg")

            # DMA chunks ordered by first use so PE starts early
            nc.sync.dma_start(out=kq[:, 0:1024], in_=kq_d[:, 0:1024])
            nc.sync.dma_start(out=kq[:, 4096:5120], in_=kq_d[:, 4096:5120])
            nc.sync.dma_start(out=mm_t[:], in_=mm_d[:])
            nc.sync.dma_start(out=kg[:], in_=kg_d[:])
            nc.sync.dma_start(out=kq[:, 1024:4096], in_=kq_d[:, 1024:4096])
            nc.sync.dma_start(out=kq[:, 5120:8192], in_=kq_d[:, 5120:8192])
            nc.sync.dma_start(out=ve[:, 0:520], in_=ve_d[:, 0:520])
            nc.sync.dma_start(out=vel[:, 0:520], in_=vel_d[:, 0:520])
            nc.sync.dma_start(out=vg[:], in_=vg_d[:])
            nc.sync.dma_start(out=ve[:, 520:2080], in_=ve_d[:, 520:2080])
            nc.sync.dma_start(out=vel[:, 520:2080], in_=vel_d[:, 520:2080])

            def kt(c0, w):
                return kq[:, c0:c0 + w]

            def qt(c0, w):
                return kq[:, 4096 + c0:4096 + c0 + w]

            def schrau(out_ap, in_ap):
                nc.vector.tensor_scalar(out=out_ap.bitcast(i16), in0=in_ap,
                                        scalar1=SA, scalar2=SB,
                                        op0=ALU.mult, op1=ALU.add)

            state = {}

            def band_mm(ps_tile, pcol, kind, idx, s):
                """One band QK matmul into ps_tile[:, pcol:pcol+w]."""
                ql = 512 * s
                if kind == "L":
                    kti = 4 * (s - 1) + idx
                    w = 128 * (idx + 1)
                    rhs = qt(ql, w)
                else:
                    kti = 4 * s + idx
                    w = 512 - 128 * idx
                    rhs = qt(ql + 128 * idx, w)
                nc.tensor.matmul(ps_tile[:, pcol:pcol + w],
                                 lhsT=kt(128 * kti, 128), rhs=rhs,
                                 start=True, stop=True)
                return w

            def qk_block(s):
                ql = 512 * s
                E = ebp.tile([128, 2816], bf16, tag="eband", name="E")
                ng = 8 * s

                pa = ps_a.tile([128, 1536], f32, tag="pa")
                pb1 = ps_b1.tile([128, 512], f32, tag="pb1")
                pb2 = ps_b2.tile([128, 512], f32, tag="pb2")
                pc = ps_c.tile([128, 512], f32, tag="pc")

                if s >= 1:
                    # pair A3: L4 @ [0:512], D1 @ [512:1024], G @ [1024:1280]
                    band_mm(pa, 0, "L", 3, s)
                    band_mm(pa, 512, "D", 0, s)
                    nc.tensor.matmul(pa[0:ng, 1024:1280], lhsT=kg[:, 0:ng],
                                     rhs=qt(ql, 256), start=True, stop=True)
                    nc.tensor.matmul(pa[64:64 + ng, 1024:1280],
                                     lhsT=kg[:, 0:ng], rhs=qt(ql + 256, 256),
                                     start=True, stop=True)
                    nc.scalar.activation(E[:, 0:1280], pa[:, 0:1280],
                                         AF.Exp, scale=SCALE)
                    # pair B1: L3 @ [0:384], D4 @ [384:512] -> E[1280:1792]
                    band_mm(pb1, 0, "L", 2, s)
                    band_mm(pb1, 384, "D", 3, s)
                    schrau(E[:, 1280:1792], pb1[:, 0:512])
                    # pair B2: L1 @ [0:128], D2 @ [128:512] -> E[1792:2304]
                    band_mm(pb2, 0, "L", 0, s)
                    band_mm(pb2, 128, "D", 1, s)
                    schrau(E[:, 1792:2304], pb2[:, 0:512])
                    # pair C: L2 @ [0:256], D3 @ [256:512] -> E[2304:2816]
                    band_mm(pc, 0, "L", 1, s)
                    band_mm(pc, 256, "D", 2, s)
                    nc.scalar.activation(E[:, 2304:2816], pc[:, 0:512],
                                         AF.Exp, scale=SCALE)
                else:
                    band_mm(pa, 512, "D", 0, s)
                    nc.scalar.activation(E[:, 512:1024], pa[:, 512:1024],
                                         AF.Exp, scale=SCALE)
                    band_mm(pb1, 384, "D", 3, s)
                    schrau(E[:, 1664:1792], pb1[:, 384:512])
                    band_mm(pb2, 128, "D", 1, s)
                    schrau(E[:, 1920:2304], pb2[:, 128:512])
                    band_mm(pc, 256, "D", 2, s)
                    nc.scalar.activation(E[:, 2560:2816], pc[:, 256:512],
                                         AF.Exp, scale=SCALE)

                # triangle masks post-exp: DVE bf16 muls for pair A3's two
                # blocks, Pool affine_selects for the remaining six.
                nc.vector.tensor_mul(E[:, 512:640], E[:, 512:640],
                                     mm_t[:, 128:256])
                if s >= 1:
                    nc.vector.tensor_mul(E[:, 384:512], E[:, 384:512],
                                         mm_t[:, 0:128])
                sels = [("D", EOFF_D[1]), ("D", EOFF_D[3]), ("D", EOFF_D[2])]
                if s >= 1:
                    sels += [("L", EOFF_L[2] + 256), ("L", EOFF_L[0]),
                             ("L", EOFF_L[1] + 128)]
                for knd, off in sels:
                    if knd == "D":
                        nc.gpsimd.affine_select(
                            out=E[:, off:off + 128], in_=E[:, off:off + 128],
                            compare_op=ALU.is_ge, fill=0.0, base=0,
                            pattern=[[1, 128]], channel_multiplier=-1)
                    else:
                        nc.gpsimd.affine_select(
                            out=E[:, off:off + 128], in_=E[:, off:off + 128],
                            compare_op=ALU.is_ge, fill=0.0, base=0,
                            pattern=[[-1, 128]], channel_multiplier=1)
                state[s] = (E, ng)

            def pv_block(s):
                E, ng = state.pop(s)
                out_ps = ps_o.tile([128, 260], f32, tag="ops")
                for t in range(4):
                    mms = []
                    if ng:
                        gp = 64 * (t // 2)
                        mms.append((E[gp:gp + ng, EG_OFF + 128 * (t % 2):
                                      EG_OFF + 128 * (t % 2) + 128],
                                    vg[gp:gp + ng, 0:65]))
                    if s >= 1:
                        for j0 in range(3, t - 1, -1):
                            kti = 4 * (s - 1) + j0
                            mms.append((E[:, EOFF_L[j0] + 128 * t:
                                          EOFF_L[j0] + 128 * t + 128],
                                        vel[:, 65 * kti:65 * kti + 65]))
                    for u in range(0, t + 1):
                        kti = 4 * s + u
                        mms.append((E[:, EOFF_D[u] + 128 * (t - u):
                                      EOFF_D[u] + 128 * (t - u) + 128],
                                    ve[:, 65 * kti:65 * kti + 65]))
                    for i, (lhsT, rhs) in enumerate(mms):
                        nc.tensor.matmul(out_ps[:, 65 * t:65 * t + 65],
                                         lhsT=lhsT, rhs=rhs,
                                         start=(i == 0), stop=(i == len(mms) - 1))
                i = PROC.index(s)
                half = i % 2
                if half == 0:
                    state[("o", i)] = osb.tile([128, 520], bf16, tag="osb",
                                               name="o_t")
                o_t = state[("o", i - half)]
                if i % 2 == 0:
                    nc.vector.tensor_copy(o_t[:, 260 * half:260 * half + 260],
                                          out_ps[:])
                else:
                    nc.scalar.copy(o_t[:, 260 * half:260 * half + 260],
                                   out_ps[:])
                if half == 1:
                    nc.sync.dma_start(out=o_d[i // 2], in_=o_t[:])
                    del state[("o", i - 1)]

            for i, s in enumerate(PROC):
                qk_block(s)
                if i >= 2:
                    pv_block(PROC[i - 2])
            pv_block(PROC[-2])
            pv_block(PROC[-1])

    nc.compile()
    return nc


_CACHE = {}


def _get_nc():
    if "nc" not in _CACHE:
        _CACHE["nc"] = _build_nc()
    return _CACHE["nc"]


def _host_inputs(Q, K, V):
    qt2 = np.ascontiguousarray(Q.T)
    kt2 = np.ascontiguousarray(K.T)
    kq = np.concatenate([kt2, qt2], axis=1).astype(ml_dtypes.bfloat16)

    ve3 = np.ones((128, 32, 65), np.float32)
    ve3[:, :, :64] = V.reshape(32, 128, 64).transpose(1, 0, 2)
    ve = ve3.reshape(128, 32 * 65).astype(ml_dtypes.bfloat16)
    ve3l = ve3.copy()
    ve3l[0::64, :, :] = 0.0  # strip global-stripe keys from lower-tile PV
    vel = ve3l.reshape(128, 32 * 65).astype(ml_dtypes.bfloat16)

    kg = np.zeros((64, 64), np.float32)
    kg[:, :56] = K[::GS, :][:56].T
    kg = kg.astype(ml_dtypes.bfloat16)
    vg = np.zeros((128, 65), np.float32)
    vg[:56, :64] = V[::GS, :][:56]
    vg[:56, 64] = 1.0
    vg[64:120] = vg[:56]
    vg = vg.astype(ml_dtypes.bfloat16)
    return dict(kq=kq, ve=ve, vel=vel, kg=kg, vg=vg)


def kernel(Q, K, V):
    global LAST_RESULT
    Q = np.ascontiguousarray(np.asarray(Q), dtype=np.float32)
    K = np.ascontiguousarray(np.asarray(K), dtype=np.float32)
    V = np.ascontiguousarray(np.asarray(V), dtype=np.float32)
    B, H, t, d = Q.shape
    assert (B, H, t, d) == (1, 8, T, D)

    kk = np.arange(128)[:, None]
    rr = np.arange(128)[None, :]
    mm = np.concatenate([(rr <= kk), (rr >= kk)],
                        axis=1).astype(ml_dtypes.bfloat16)

    nc = _get_nc()
    in_maps = []
    for h in range(8):
        m = _host_inputs(Q[0, h], K[0, h], V[0, h])
        m["mm"] = mm
        in_maps.append(m)

    res = run_bass_kernel_spmd(nc, in_maps, list(range(8)), trace=TRACE)
    LAST_RESULT = res

    out = np.empty((1, 8, T, D), np.float32)
    for h in range(8):
        O = res.results[h]["o"].astype(np.float32)  # [4, 128, 520]
        for i, s in enumerate(PROC):
            o4 = O[i // 2, :, 260 * (i % 2):260 * (i % 2) + 260].reshape(128, 4, 65)
            for tq in range(4):
                blk = o4[:, tq, :64] / o4[:, tq, 64:65]
                out[0, h, 512 * s + 128 * tq:512 * s + 128 * (tq + 1), :] = blk
    return out


# revision 22
# speedup vs baseline: 1.0401x; 1.0401x over previous
"""Local+global sparse attention (T=4096, D=64, window=512, global stride 64)
for Trainium2, one head per NeuronCore (B*H = 8 = n_cores).

Per-head structure (hardcoded for T=4096, D=64), for each of 8 query
superblocks (sb) of 512 queries, scores S^T in [k=128 part, q free] tiles:
  lower tiles j0=0..3 : keys [512(s-1)+128*j0, +128), queries [0, 128*(j0+1))
  diag  tiles u =0..3 : keys [512 s +128*u, +128),    queries [128*u, 512)
  global tile         : ng=8s stride-64 keys k < 512 s, packed two q-halves
                        on partitions [0:ng] / [64:64+ng], 256 q cols each.
Band QK matmuls write shared multi-bank PSUM pair tiles so one wide exp
instruction covers two tiles.  exp is split: ScalarE native Exp for pairs
A (L4+D1), C (L2+D3), and the global tile; DVE runs Schraudolph int16 exp
(round(y*128/ln2 + B) bitcast to bf16) for pairs B (L3+D2) and D (L1+D4).
Causal/window-edge triangles are zeroed post-exp by affine_selects (Pool x7,
DVE x1).  Global-stripe keys inside lower tiles are excluded via ve_low (PV
rhs with those key rows zeroed); their contribution comes from the global
tile.  PV is transposed: out[q=128 part, 65] += E[k, qslice].T @ ve[k, 65]
(65th column of ones = softmax denominator).  Host divides by denominator.
"""

import sys

sys.path.insert(0, "/opt/trn_rl_repo")

from contextlib import ExitStack

import numpy as np
import ml_dtypes

import concourse.bass as bass
import concourse.mybir as mybir
import concourse.tile as tile
from concourse import bacc
from concourse.bass_utils import run_bass_kernel_spmd

f32 = mybir.dt.float32
bf16 = mybir.dt.bfloat16
i16 = mybir.dt.int16
AF = mybir.ActivationFunctionType
ALU = mybir.AluOpType

T, D = 4096, 64
W, GS = 512, 64
NSB = T // 512
SCALE = 1.0 / 8.0
# Schraudolph exp in bf16: round((s*SCALE)*(128/ln2) + SB) as int16 == bf16 bits
SA = (128.0 / float(np.log(2.0))) * SCALE
SB = 16250.40

# E-region offsets (bf16 cols) matching the PSUM pair layout:
# [A3: L4|D1|G 0:1280] [B1: L3|D4 1280:1792] [B2: D2|L1 1792:2304]
# [C: L2|D3 2304:2816]
EOFF_L = {3: 0, 2: 1280, 1: 2304, 0: 1792}
EOFF_D = {0: 512, 1: 1920, 2: 2560, 3: 1664}
EG_OFF = 1024

# superblock processing order: lightest (s=0) drains last
PROC = [1, 2, 3, 4, 5, 6, 7, 0]

TRACE = False
LAST_RESULT = None


def _build_nc():
    nc = bacc.Bacc("TRN2", target_bir_lowering=False, debug=False, num_devices=8)
    kq_d = nc.dram_tensor("kq", [64, 8192], bf16, kind="ExternalInput")
    mm_d = nc.dram_tensor("mm", [128, 256], bf16, kind="ExternalInput")
    ve_d = nc.dram_tensor("ve", [128, 32 * 65], bf16, kind="ExternalInput")
    vel_d = nc.dram_tensor("vel", [128, 32 * 65], bf16, kind="ExternalInput")
    kg_d = nc.dram_tensor("kg", [64, 64], bf16, kind="ExternalInput")
    vg_d = nc.dram_tensor("vg", [128, 65], bf16, kind="ExternalInput")
    o_d = nc.dram_tensor("o", [4, 128, 520], bf16, kind="ExternalOutput")

    with tile.TileContext(nc) as tc:
        with ExitStack() as ctx:
            const = ctx.enter_context(tc.tile_pool(name="const", bufs=1))
            ebp = ctx.enter_context(tc.tile_pool(name="ebp", bufs=3))
            egp = ctx.enter_context(tc.tile_pool(name="egp", bufs=3))
            osb = ctx.enter_context(tc.tile_pool(name="osb", bufs=2))
            ps_a = ctx.enter_context(tc.tile_pool(name="ps_a", bufs=1, space="PSUM"))
            ps_b1 = ctx.enter_context(tc.tile_pool(name="ps_b1", bufs=1, space="PSUM"))
            ps_b2 = ctx.enter_context(tc.tile_pool(name="ps_b2", bufs=1, space="PSUM"))
            ps_c = ctx.enter_context(tc.tile_pool(name="ps_c", bufs=1, space="PSUM"))
            ps_o = ctx.enter_context(tc.tile_pool(name="ps_o", bufs=2, space="PSUM"))

            kq = const.tile([64, 8192], bf16, tag="kq")
            mm_t = const.tile([128, 256], bf16, tag="mm")
            ve = const.tile([128, 32 * 65], bf16, tag="ve")
            vel = const.tile([128, 32 * 65], bf16, tag="vel")
            kg = const.tile([64, 64], bf16, tag="kg")
            vg = const.tile([128, 65], bf16, tag="vg")

            # DMA chunks ordered by first use so PE starts early
            nc.sync.dma_start(out=kq[:, 0:1024], in_=kq_d[:, 0:1024])
            nc.sync.dma_start(out=kq[:, 4096:5120], in_=kq_d[:, 4096:5120])
            nc.sync.dma_start(out=kg[:], in_=kg_d[:])
            nc.sync.dma_start(out=kq[:, 1024:2560], in_=kq_d[:, 1024:2560])
            nc.sync.dma_start(out=kq[:, 5120:6656], in_=kq_d[:, 5120:6656])
            nc.sync.dma_start(out=mm_t[:], in_=mm_d[:])
            nc.sync.dma_start(out=ve[:, 0:520], in_=ve_d[:, 0:520])
            nc.sync.dma_start(out=vel[:, 0:520], in_=vel_d[:, 0:520])
            nc.sync.dma_start(out=vg[:], in_=vg_d[:])
            nc.sync.dma_start(out=kq[:, 2560:4096], in_=kq_d[:, 2560:4096])
            nc.sync.dma_start(out=kq[:, 6656:8192], in_=kq_d[:, 6656:8192])
            nc.sync.dma_start(out=ve[:, 520:2080], in_=ve_d[:, 520:2080])
            nc.sync.dma_start(out=vel[:, 520:2080], in_=vel_d[:, 520:2080])

            def kt(c0, w):
                return kq[:, c0:c0 + w]

            def qt(c0, w):
                return kq[:, 4096 + c0:4096 + c0 + w]

            def schrau(out_ap, in_ap):
                nc.vector.tensor_scalar(out=out_ap.bitcast(i16), in0=in_ap,
                                        scalar1=SA, scalar2=SB,
                                        op0=ALU.mult, op1=ALU.add)

            state = {}

            def band_mm(ps_tile, pcol, kind, idx, s):
                """One band QK matmul into ps_tile[:, pcol:pcol+w]."""
                ql = 512 * s
                if kind == "L":
                    kti = 4 * (s - 1) + idx
                    w = 128 * (idx + 1)
                    rhs = qt(ql, w)
                else:
                    kti = 4 * s + idx
                    w = 512 - 128 * idx
                    rhs = qt(ql + 128 * idx, w)
                nc.tensor.matmul(ps_tile[:, pcol:pcol + w],
                                 lhsT=kt(128 * kti, 128), rhs=rhs,
                                 start=True, stop=True)
                return w

            def qk_block(s):
                ql = 512 * s
                E = ebp.tile([128, 2816], bf16, tag="eband", name="E")
                ng = 8 * s

                pa = ps_a.tile([128, 1536], f32, tag="pa")
                pb1 = ps_b1.tile([128, 512], f32, tag="pb1")
                pb2 = ps_b2.tile([128, 512], f32, tag="pb2")
                pc = ps_c.tile([128, 512], f32, tag="pc")

                if s >= 1:
                    # pair A3: L4 @ [0:512], D1 @ [512:1024], G @ [1024:1280]
                    band_mm(pa, 0, "L", 3, s)
                    band_mm(pa, 512, "D", 0, s)
                    nc.tensor.matmul(pa[0:ng, 1024:1280], lhsT=kg[:, 0:ng],
                                     rhs=qt(ql, 256), start=True, stop=True)
                    nc.tensor.matmul(pa[64:64 + ng, 1024:1280],
                                     lhsT=kg[:, 0:ng], rhs=qt(ql + 256, 256),
                                     start=True, stop=True)
                    nc.scalar.activation(E[:, 0:1280], pa[:, 0:1280],
                                         AF.Exp, scale=SCALE)
                    # pair B1: L3 @ [0:384], D4 @ [384:512] -> E[1280:1792]
                    band_mm(pb1, 0, "L", 2, s)
                    band_mm(pb1, 384, "D", 3, s)
                    schrau(E[:, 1280:1792], pb1[:, 0:512])
                    # pair B2: L1 @ [0:128], D2 @ [128:512] -> E[1792:2304]
                    band_mm(pb2, 0, "L", 0, s)
                    band_mm(pb2, 128, "D", 1, s)
                    schrau(E[:, 1792:2304], pb2[:, 0:512])
                    # pair C: L2 @ [0:256], D3 @ [256:512] -> E[2304:2816]
                    band_mm(pc, 0, "L", 1, s)
                    band_mm(pc, 256, "D", 2, s)
                    nc.scalar.activation(E[:, 2304:2816], pc[:, 0:512],
                                         AF.Exp, scale=SCALE)
                else:
                    band_mm(pa, 512, "D", 0, s)
                    nc.scalar.activation(E[:, 512:1024], pa[:, 512:1024],
                                         AF.Exp, scale=SCALE)
                    band_mm(pb1, 384, "D", 3, s)
                    schrau(E[:, 1664:1792], pb1[:, 384:512])
                    band_mm(pb2, 128, "D", 1, s)
                    schrau(E[:, 1920:2304], pb2[:, 128:512])
                    band_mm(pc, 256, "D", 2, s)
                    nc.scalar.activation(E[:, 2560:2816], pc[:, 256:512],
                                         AF.Exp, scale=SCALE)

                # triangle masks post-exp: DVE bf16 muls for pair A3's two
                # blocks, Pool affine_selects for the remaining six.
                nc.vector.tensor_mul(E[:, 512:640], E[:, 512:640],
                                     mm_t[:, 128:256])
                if s >= 1:
                    nc.vector.tensor_mul(E[:, 384:512], E[:, 384:512],
                                         mm_t[:, 0:128])
                sels = [("D", EOFF_D[1]), ("D", EOFF_D[3]), ("D", EOFF_D[2])]
                if s >= 1:
                    sels += [("L", EOFF_L[2] + 256), ("L", EOFF_L[0]),
                             ("L", EOFF_L[1] + 128)]
                for knd, off in sels:
                    if knd == "D":
                        nc.gpsimd.affine_select(
                            out=E[:, off:off + 128], in_=E[:, off:off + 128],
                            compare_op=ALU.is_ge, fill=0.0, base=0,
                            pattern=[[1, 128]], channel_multiplier=-1)
                    else:
                        nc.gpsimd.affine_select(
                            out=E[:, off:off + 128], in_=E[:, off:off + 128],
                            compare_op=ALU.is_ge, fill=0.0, base=0,
                            pattern=[[-1, 128]], channel_multiplier=1)
                state[s] = (E, ng)

            def pv_block(s):
                E, ng = state.pop(s)
                out_ps = ps_o.tile([128, 260], f32, tag="ops")
                for t in range(4):
                    mms = []
                    if ng:
                        gp = 64 * (t // 2)
                        mms.append((E[gp:gp + ng, EG_OFF + 128 * (t % 2):
                                      EG_OFF + 128 * (t % 2) + 128],
                                    vg[gp:gp + ng, 0:65]))
                    if s >= 1:
                        for j0 in range(3, t - 1, -1):
                            kti = 4 * (s - 1) + j0
                            mms.append((E[:, EOFF_L[j0] + 128 * t:
                                          EOFF_L[j0] + 128 * t + 128],
                                        vel[:, 65 * kti:65 * kti + 65]))
                    for u in range(0, t + 1):
                        kti = 4 * s + u
                        mms.append((E[:, EOFF_D[u] + 128 * (t - u):
                                      EOFF_D[u] + 128 * (t - u) + 128],
                                    ve[:, 65 * kti:65 * kti + 65]))
                    for i, (lhsT, rhs) in enumerate(mms):
                        nc.tensor.matmul(out_ps[:, 65 * t:65 * t + 65],
                                         lhsT=lhsT, rhs=rhs,
                                         start=(i == 0), stop=(i == len(mms) - 1))
                i = PROC.index(s)
                half = i % 2
                if half == 0:
                    state[("o", i)] = osb.tile([128, 520], bf16, tag="osb",
                                               name="o_t")
                o_t = state[("o", i - half)]
                nc.vector.tensor_copy(o_t[:, 260 * half:260 * half + 260],
                                      out_ps[:])
                if half == 1:
                    nc.sync.dma_start(out=o_d[i // 2], in_=o_t[:])
                    del state[("o", i - 1)]

            for i, s in enumerate(PROC):
                qk_block(s)
                if i >= 2:
                    pv_block(PROC[i - 2])
            pv_block(PROC[-2])
            pv_block(PROC[-1])

    nc.compile()
    return nc


_CACHE = {}


def _get_nc():
    if "nc" not in _CACHE:
        _CACHE["nc"] = _build_nc()
    return _CACHE["nc"]


def _host_inputs(Q, K, V):
    qt2 = np.ascontiguousarray(Q.T)
    kt2 = np.ascontiguousarray(K.T)
    kq = np.concatenate([kt2, qt2], axis=1).astype(ml_dtypes.bfloat16)

    ve3 = np.ones((128, 32, 65), np.float32)
    ve3[:, :, :64] = V.reshape(32, 128, 64).transpose(1, 0, 2)
    ve = ve3.reshape(128, 32 * 65).astype(ml_dtypes.bfloat16)
    ve3l = ve3.copy()
    ve3l[0::64, :, :] = 0.0  # strip global-stripe keys from lower-tile PV
    vel = ve3l.reshape(128, 32 * 65).astype(ml_dtypes.bfloat16)

    kg = np.zeros((64, 64), np.float32)
    kg[:, :56] = K[::GS, :][:56].T
    kg = kg.astype(ml_dtypes.bfloat16)
    vg = np.zeros((128, 65), np.float32)
    vg[:56, :64] = V[::GS, :][:56]
    vg[:56, 64] = 1.0
    vg[64:120] = vg[:56]
    vg = vg.astype(ml_dtypes.bfloat16)
    return dict(kq=kq, ve=ve, vel=vel, kg=kg, vg=vg)


def kernel(Q, K, V):
    global LAST_RESULT
    Q = np.ascontiguousarray(np.asarray(Q), dtype=np.float32)
    K = np.ascontiguousarray(np.asarray(K), dtype=np.float32)
    V = np.ascontiguousarray(np.asarray(V), dtype=np.float32)
    B, H, t, d = Q.shape
    assert (B, H, t, d) == (1, 8, T, D)

    kk = np.arange(128)[:, None]
    rr = np.arange(128)[None, :]
    mm = np.concatenate([(rr <= kk), (rr >= kk)],
                        axis=1).astype(ml_dtypes.bfloat16)

    nc = _get_nc()
    in_maps = []
    for h in range(8):
        m = _host_inputs(Q[0, h], K[0, h], V[0, h])
        m["mm"] = mm
        in_maps.append(m)

    res = run_bass_kernel_spmd(nc, in_maps, list(range(8)), trace=TRACE)
    LAST_RESULT = res

    out = np.empty((1, 8, T, D), np.float32)
    for h in range(8):
        O = res.results[h]["o"].astype(np.float32)  # [4, 128, 520]
        for i, s in enumerate(PROC):
            o4 = O[i // 2, :, 260 * (i % 2):260 * (i % 2) + 260].reshape(128, 4, 65)
            for tq in range(4):
                blk = o4[:, tq, :64] / o4[:, tq, 64:65]
                out[0, h, 512 * s + 128 * tq:512 * s + 128 * (tq + 1), :] = blk
    return out
